# revision 15
# baseline (speedup 1.0000x reference)
"""Trainium2 Bass kernel for nn_EdgeClassify (gnn_message_passing), v2.

Reference computation (B=64, S=2048, D=1024, A=13, NB=4):
    red = einsum('bsd,ad->bsa', e_output, W1) + b1      # [B,S,A]
    f   = swapaxes(red[:, :A, :], 1, 2)                 # [B,A,A]  (only s<A used!)
    ga  = einsum('bia,na->bin', f, Wf[:, :A])
    gb  = einsum('bia,na->bin', f, Wf[:, A:])
    out[b,i,j,n] = ga[b,min(i,j),n] + gb[b,max(i,j),n] + bf[n], 0 on diagonal

Only e_output[:, :A, :] affects the output. Device math per core (8
batches, data parallel over B), all matmul operands bf16:
    Z  [104(b,m), 13(i)]  = sum_d x[(b,m), d] * W1[i, d]     (8 matmuls)
    G  [13(i), 64(s,b,n)] = Z.T @ W_blockdiag                (1 matmul)
    O  [32(b,n), 169(ij)] = Ga.T @ [M1T; CM] + Gb.T @ M2T    (2 matmuls,
                            accumulated; CM rows fold all b1/bf biases via
                            4 constant indicator rows in the lhsT)
PSUM->SBUF staging copies run on DVE (lowest post-op access latency of
the engines whose copies lower on this device path; gpsimd tensor_copy
and Activation copies both fail in this environment). Output goes out
via a SWDGE scatter-add whose descriptors are prepared during the input
DMA wait and fired with trigger_dma once the staging copy lands - this
skips both the 625ns HWDGE issue and the 650ns DGE->DMA delay on the
critical tail. The scatter moves bf16
(host converts back to fp32) into a pre-zeroed [32,256]-bf16 DRAM
buffer (256 = 169 padded to a 256B-multiple row stride; 240 declared
rows keep every idx value in bounds). The framework's entry/exit all-engine
barriers are stripped (every cross-engine dependency is explicitly
semaphored) and the first input DMA is hoisted above SP's entry branch,
together starting the input transfer ~270ns earlier.

Timeline (per core, TimelineSim): input DMA issue+latency 1275ns ->
x transfer 667ns -> DMA-sem prop 925ns -> mm1 -> Z copy -> mm2 ->
G copy -> mm3a/b -> out copy -> trigger+transfer ~80ns -> DMA-sem
prop 925ns. Total ~5805ns (baseline 8227ns).
"""

import os

import numpy as np

os.environ.setdefault("BASS_NEVER_TRACE", "1")

import concourse.bass as bass
import concourse.bacc as bacc
import concourse.mybir as mybir
from concourse.bass_utils import run_bass_kernel_spmd
from ml_dtypes import bfloat16

B, S, D, A, NB = 64, 2048, 1024, 13, 4
NCORES = 8
BPC = B // NCORES          # 8 batches per core
BM = BPC * A               # 104 (b, m) rows per core
AA = A * A                 # 169
NCH = D // 128             # 8 contraction chunks
OROW = 256                 # padded out row (bf16): 169 -> 256 (512B, 256B-aligned)
ODROWS = 240               # out DRAM rows: 32 used; padded so iota idx
                           # values (p + 16s, p<128) stay in bounds
F32 = mybir.dt.float32
BF16 = mybir.dt.bfloat16
I16 = mybir.dt.int16

# blob column offsets (bf16 columns)
W1C = 0                    # w1t: chunk c at cols c*13, row p = d%128
XC = NCH * A               # 104: x chunks (c-major, 104 cols each)
IDXC = XC + NCH * BM       # 936: scatter idx bits (2 cols, int16-as-bf16)
D1END = IDXC + 2           # 938: end of DMA1 (w1t + x + idx)
WABC = D1END               # 938: block-diag [104, 64] both Wf halves
G2C = WABC + 64            # 1002: g2s lhsT [17, 64]; rows 13:17 host consts
M1C = G2C + 64             # 1066: [17, 169]: rows 0:13 M1T, 13:17 cm
M2C = M1C + AA             # 1235: [13, 169]: M2T
COLS = M2C + AA            # 1404
GR = A + NB                # 17: g2s rows (13 data + 4 bias indicators)

_COMPILED = {}


def build_program(out_mode="scatter", nwarm=7, warm_cols=256,
                  copy_eng="dve", final_wait=True, act_split=False,
                  strip_barriers=True) -> bass.Bass:
    nc = bacc.Bacc("TRN2", target_bir_lowering=False, debug=False,
                   num_devices=NCORES)

    blob_d = nc.declare_dram_parameter("blob", [128, COLS], BF16, isOutput=False)
    out_d = nc.declare_dram_parameter("out", [ODROWS, OROW], BF16, isOutput=True)

    from contextlib import ExitStack
    with ExitStack() as es:
        blob = es.enter_context(nc.sbuf_tensor([128, COLS], BF16))
        zs = es.enter_context(nc.sbuf_tensor([BM, A], BF16))
        idxt = es.enter_context(nc.sbuf_tensor([128, 2], I16))
        outs = es.enter_context(nc.sbuf_tensor([128, 1, OROW], BF16))
        wp = es.enter_context(nc.psum_tensor([1, warm_cols], F32))
        zp = es.enter_context(nc.psum_tensor([BM, A], F32))
        gp = es.enter_context(nc.psum_tensor([A, 64], F32))
        op = es.enter_context(nc.psum_tensor([BPC * NB, AA], F32))
        (dsem1, dsem2, zsem, dsem3, pm, psem, isem, s1, sza, s2, sc, s3,
         sv) = (es.enter_context(nc.semaphore(n)) for n in (
            "dsem1", "dsem2", "zsem", "dsem3", "pm", "psem", "isem", "s1",
            "sza", "s2", "sc", "s3", "sv"))
        block = es.enter_context(nc.Block())
        @block.sync
        def _(sync):
            # w1t + x + scatter idx first: gates stage 1 (and the scatter
            # prep); consts transfer while stage 1's data is still in flight
            sync.dma_start(blob[:, 0:D1END], blob_d[:, 0:D1END]).then_inc(
                dsem1, 16)
            sync.dma_start(blob[:, D1END:COLS], blob_d[:, D1END:COLS]).then_inc(
                dsem2, 16)
            if out_mode in ("scatter", "scatter_direct"):
                # pre-zero the DRAM output (scatter-add needs a clean base).
                # Source rows 32:64 (memset zeros the whole tile): the copies
                # later write rows 0:32, so this read never conflicts with
                # them and they need no zsem ordering.
                sync.dma_start(out_d[0:BPC * NB, :], outs[BPC * NB:2 * BPC * NB,
                                                          0, :]
                               ).wait_op(pm, 1, "sem-ge").then_inc(zsem, 16)
            else:
                sync.dma_start(out_d[0:BPC * NB, :], outs[0:BPC * NB, 0, :]
                               ).wait_op(sv, 2 if act_split else 1,
                                         "sem-ge").then_inc(dsem3, 16)

        @block.gpsimd
        def _(gpsimd):
            if out_mode == "scatter" and copy_eng in ("pool", "pool_blobidx"):
                if copy_eng == "pool":
                    # idx on-device (p + 16s): frees the scatter prep from
                    # the input-DMA wait, so Pool's engine is idle in time
                    # for the PSUM->SBUF staging copies below
                    nc.gpsimd.iota(idxt[:, :], pattern=[[16, 2]], base=0,
                                   channel_multiplier=1).then_inc(isem, 1)
                    prep_wait, prep_val = isem, 1
                    idxs_ap = idxt[:, :]
                else:
                    prep_wait, prep_val = dsem1, 16
                    idxs_ap = blob[0:128, IDXC:IDXC + 2].bitcast(I16)
                nc.gpsimd.dma_scatter_add(
                    out_ap=out_d[:, :],
                    in_ap=outs[:, :, :],
                    idxs_ap=idxs_ap,
                    num_idxs=BPC * NB,
                    num_idxs_reg=BPC * NB,
                    elem_size=OROW,
                    prepare_only=True,
                    sem=dsem3,
                ).wait_op(prep_wait, prep_val, "sem-ge").then_inc(psem, 1)
                gpsimd.memset(outs[:, :, :], 0.0).then_inc(pm, 1)
                # staging copies: gpsimd has no post-op access latency (vs
                # DVE's +125ns) and the trigger below waits on a same-engine
                # semaphore
                nc.gpsimd.tensor_copy(zs[:], zp[:]).wait_op(
                    s1, 1, "sem-ge").then_inc(sza, 1)
                nc.gpsimd.tensor_copy(blob[0:A, G2C:G2C + 64], gp[:]).wait_op(
                    s2, 1, "sem-ge").then_inc(sc, 1)
                gpsimd.wait_ge(zsem, 16)
                nc.gpsimd.tensor_copy(outs[0:BPC * NB, 0, 0:AA], op[:]).wait_op(
                    s3, 1, "sem-ge").then_inc(sv, 1)
                gpsimd.wait_ge(psem, 1)
                nc.gpsimd.trigger_dma(count=1).wait_op(sv, 1, "sem-ge")
            elif out_mode == "scatter":
                gpsimd.memset(outs[:, :, :], 0.0).then_inc(pm, 1)
                nc.gpsimd.dma_scatter_add(
                    out_ap=out_d[:, :],
                    in_ap=outs[:, :, :],
                    idxs_ap=blob[0:128, IDXC:IDXC + 2].bitcast(I16),
                    num_idxs=BPC * NB,
                    num_idxs_reg=BPC * NB,
                    elem_size=OROW,
                    prepare_only=True,
                    sem=dsem3,
                ).wait_op(dsem1, 16, "sem-ge").then_inc(psem, 1)
                gpsimd.wait_ge(psem, 1)
                gpsimd.wait_ge(zsem, 16)
                nc.gpsimd.trigger_dma(count=1).wait_op(sv, 2 if act_split else 1, "sem-ge")
            elif out_mode == "scatter_direct":
                gpsimd.memset(outs[:, :, :], 0.0).then_inc(pm, 1)
                gpsimd.wait_ge(zsem, 16)
                nc.gpsimd.dma_scatter_add(
                    out_ap=out_d[:, :],
                    in_ap=outs[:, :, :],
                    idxs_ap=blob[0:128, IDXC:IDXC + 2].bitcast(I16),
                    num_idxs=BPC * NB,
                    num_idxs_reg=BPC * NB,
                    elem_size=OROW,
                ).wait_op(sv, 2 if act_split else 1, "sem-ge").then_inc(dsem3, 16)

        @block.tensor
        def _(tensor):
            # warm-up matmuls on (garbage) blob data keep the PE p-state
            # ramped while the input DMA is in flight
            for _ in range(nwarm):
                nc.tensor.matmul(wp[:], blob[:, 0:1], blob[:, 0:warm_cols],
                                 start=True, stop=True)
            # stage 1: Z[(b,m), i] = sum_d x[(b,m), d] * W1[i, d]
            for c in range(NCH):
                mm = nc.tensor.matmul(
                    zp[:],
                    blob[:, XC + c * BM:XC + (c + 1) * BM],  # lhsT [128, 104]
                    blob[:, W1C + c * A:W1C + (c + 1) * A],  # rhs  [128, 13]
                    start=(c == 0),
                    stop=(c == NCH - 1),
                )
                if c == 0:
                    mm.wait_op(dsem1, 16, "sem-ge")
            mm.then_inc(s1, 1)
            # stage 2: G[i, (side,b,n)] = Z.T @ W_blockdiag(both halves)
            # (consts wait is standalone: dsem2 fires well before sza)
            tensor.wait_ge(dsem2, 16)
            nc.tensor.matmul(
                gp[:], zs[:], blob[0:BM, WABC:WABC + 64],
                start=True, stop=True,
            ).wait_op(sza, 1, "sem-ge").then_inc(s2, 1)
            # stage 3: O = Ga.T @ [M1T; CM] + Gb.T @ M2T  (accumulate in op)
            nc.tensor.matmul(
                op[:], blob[0:GR, G2C:G2C + 32], blob[0:GR, M1C:M1C + AA],
                start=True, stop=False, skip_group_check=True,
            ).wait_op(sc, 1, "sem-ge")
            nc.tensor.matmul(
                op[:], blob[0:A, G2C + 32:G2C + 64], blob[0:A, M2C:M2C + AA],
                start=False, stop=True, skip_group_check=True,
            ).then_inc(s3, 1)

        # out-copy column split: DVE takes cols 0:OSPL, Act takes the rest;
        # tuned so both engines' (processing + access-ack) latencies finish
        # together, ~30ns sooner than DVE alone. Small copies stay DVE-only
        # (Act's 370ns access init dwarfs them).
        OSPL = 139 if act_split else AA

        if copy_eng == "dve":
            @block.vector
            def _(vector):
                nc.vector.tensor_copy(zs[:], zp[:]).wait_op(
                    s1, 1, "sem-ge").then_inc(sza, 1)
                nc.vector.tensor_copy(blob[0:A, G2C:G2C + 64], gp[:]).wait_op(
                    s2, 1, "sem-ge").then_inc(sc, 1)
                # order the outs write after Pool's memset (fires ~340ns in)
                if out_mode == "scatter":
                    vector.wait_ge(pm, 1)
                nc.vector.tensor_copy(outs[0:BPC * NB, 0, 0:OSPL],
                                      op[:, 0:OSPL]).wait_op(
                    s3, 1, "sem-ge").then_inc(sv, 1)

            if act_split:
                @block.scalar
                def _(scalar):
                    # order the outs write after Pool's memset (fires ~340ns
                    # in, long before this engine's act-table load completes)
                    if out_mode == "scatter":
                        scalar.wait_ge(pm, 1)
                    nc.scalar.copy(outs[0:BPC * NB, 0, OSPL:AA],
                                   op[:, OSPL:AA]).wait_op(
                        s3, 1, "sem-ge").then_inc(sv, 1)

    if final_wait:
        # SP EventSemaphore costs 25ns after the sem resolves; cheaper
        # waiters don't exist (a no-op trigger_dma would be 0-cost in the
        # model but the executor/ucode reject an empty-FIFO trigger)
        nc.sync.wait_ge(dsem3, 16)

    _strip_dead_const_inits(nc)
    if strip_barriers:
        _strip_barriers(nc)
    _hoist_first_dma(nc)
    nc.finalize()
    return nc


def _hoist_first_dma(nc):
    """Move SP's first DMACopy from its body block into `main`, ahead of the
    UnconditionalBranch, so the input DMA issues ~50ns earlier."""
    import concourse.mybir as mb
    fn = nc.m.functions[0]
    blocks = {b.name: b for b in fn.blocks}
    main = fn.blocks[0]
    sp = mb.EngineType.SP
    br_i = next((k for k, i in enumerate(main.instructions)
                 if i.engine == sp
                 and type(i).__name__ == "InstUnconditionalBranch"), None)
    if br_i is None:
        return
    target = main.instructions[br_i].target
    body = blocks.get(target)
    if body is None or not body.instructions:
        return
    first = body.instructions[0]
    if type(first).__name__ != "InstDMACopy" or first.engine != sp:
        return
    body.instructions = body.instructions[1:]
    main.instructions = (main.instructions[:br_i] + [first]
                         + main.instructions[br_i:])


def _strip_barriers(nc):
    """Remove the framework's entry/exit all-engine barriers (Drain +
    barrier_* EventSemaphore per engine). Every cross-engine dependency in
    this program is ordered by an explicit semaphore, so the barriers only
    delay the first DMA by ~220ns. Exit Drains are also dropped; the final
    SP wait on the output-DMA semaphore keeps the program alive."""
    barrier_sems = set()
    for name, inst in nc.inst_map.items():
        if name.startswith("barrier_"):
            si = getattr(inst, "sync_info", None)
            if si is not None:
                for w in (si.on_wait or []):
                    barrier_sems.add(w.id)
                for u in (si.on_update or []):
                    barrier_sems.add(u.id)
    dead = set()
    for name, inst in nc.inst_map.items():
        tname = type(inst).__name__
        if name.startswith("barrier_"):
            dead.add(name)
        elif tname == "InstDrain":
            si = getattr(inst, "sync_info", None)
            refs = set()
            if si is not None:
                refs = {w.id for w in (si.on_wait or [])} | {
                    u.id for u in (si.on_update or [])}
            if refs <= barrier_sems:
                dead.add(name)
    if not dead:
        return
    for f in nc.m.functions:
        for b in f.blocks:
            b.instructions = [i for i in b.instructions if i.name not in dead]


def _strip_dead_const_inits(nc):
    """Drop preamble memsets that initialize Bass's lazy scratch constants
    when nothing in the program reads them (starts the first DMA earlier)."""
    read = set()
    inits = {}
    for name, inst in nc.inst_map.items():
        for ap in (getattr(inst, "ins", None) or []):
            mr = getattr(ap, "memref", "")
            if isinstance(mr, str) and mr.startswith("const-"):
                read.add(mr)
        if type(inst).__name__ == "InstMemset":
            outs = getattr(inst, "outs", None)
            if outs:
                mr = getattr(outs[0], "memref", "")
                if isinstance(mr, str) and mr.startswith("const-"):
                    inits.setdefault(mr, []).append(name)
    dead = {n for mr, names in inits.items() if mr not in read for n in names}
    if not dead:
        return
    for f in nc.m.functions:
        for b in f.blocks:
            b.instructions = [i for i in b.instructions if i.name not in dead]


def _host_consts(W1, b1, Wf, bf):
    """Host-precomputed constant blob columns (everything except x)."""
    Wa, Wb = Wf[:, :A], Wf[:, A:]
    cb = np.zeros((128, COLS), np.float32)

    # w1t: chunk c at cols c*13: w1t[p, c*13+i] = W1[i, c*128+p]
    cb[:, W1C:W1C + NCH * A] = (
        W1.T.reshape(NCH, 128, A).transpose(1, 0, 2).reshape(128, NCH * A)
    )

    # wab block-diag [104, 64]: rows (b,m), cols side*32 + b*4 + n
    for b in range(BPC):
        cb[b * A:(b + 1) * A, WABC + b * NB:WABC + (b + 1) * NB] = Wa.T
        cb[b * A:(b + 1) * A,
           WABC + 32 + b * NB:WABC + 32 + (b + 1) * NB] = Wb.T

    # g2s const rows 13:17: indicator [n == k] at col side*32 + b*4 + n
    for k in range(NB):
        for side in range(2):
            for b in range(BPC):
                cb[A + k, G2C + side * 32 + b * NB + k] = 1.0

    idx = np.arange(A)
    I, J = np.meshgrid(idx, idx, indexing="ij")
    offd = (I != J).astype(np.float32).reshape(-1)
    mn, mx = np.minimum(I, J).reshape(-1), np.maximum(I, J).reshape(-1)
    m1t = np.zeros((A, AA), np.float32)
    m2t = np.zeros((A, AA), np.float32)
    cols = np.arange(AA)
    m1t[mn, cols] = offd
    m2t[mx, cols] = offd
    cb[0:A, M1C:M1C + AA] = m1t
    cb[0:A, M2C:M2C + AA] = m2t
    # cm rows 13:17 of the M1 weight: fold b1/bf biases
    sa, sb = Wa.sum(1), Wb.sum(1)
    cm = (bf[:, None] + np.outer(sa, b1[mn]) + np.outer(sb, b1[mx])) * offd[None, :]
    cb[A:GR, M1C:M1C + AA] = cm

    cbf = cb.astype(bfloat16)

    # scatter idx: [128, 2] int16, idx j at [j%16, j//16]. Only rows 0:16
    # are decoded; pad rows with 0 (in-bounds, and 0x0000 is not a bf16 NaN,
    # which -1 = 0xFFFF would be)
    idx16 = np.zeros((128, 2), np.int16)
    for j in range(BPC * NB):
        idx16[j % 16, j // 16] = j
    cbf[:, IDXC:IDXC + 2] = idx16.view(bfloat16)
    return cbf


def _probe_batches(e_output, W1, b1, Wf, bf, batches):
    """Host-side fp32 recompute of whole batches - guards against transient
    device glitches (O(1) corruption; bf16 noise is ~5e-3)."""
    Wa, Wb = Wf[:, :A], Wf[:, A:]
    wab = np.concatenate([Wa, Wb], axis=0).T                  # [13, 8]
    idx = np.arange(A)
    I, J = np.meshgrid(idx, idx, indexing="ij")
    offd = (I != J).astype(np.float32).reshape(-1)
    mn, mx = np.minimum(I, J).reshape(-1), np.maximum(I, J).reshape(-1)
    m1t = np.zeros((A, AA), np.float32)
    m2t = np.zeros((A, AA), np.float32)
    cols = np.arange(AA)
    m1t[mn, cols] = offd
    m2t[mx, cols] = offd
    sa, sb = Wa.sum(1), Wb.sum(1)
    cm = (bf[:, None] + np.outer(sa, b1[mn]) + np.outer(sb, b1[mx])) * offd[None, :]
    out = np.empty((len(batches), A, A, NB), np.float32)
    for k, b in enumerate(batches):
        zb = e_output[b, :A, :] @ W1.T                        # [13(m), 13(i)]
        g = zb.T @ wab                                        # [13(i), 8]
        ob = g[:, :NB].T @ m1t + g[:, NB:].T @ m2t + cm       # [4, 169]
        out[k] = ob.T.reshape(A, A, NB)
    return out


def kernel(e_output, W1, b1, Wf, bf, max_atoms):
    assert int(max_atoms) == A
    e_output = np.asarray(e_output, dtype=np.float32)
    W1 = np.asarray(W1, dtype=np.float32)
    b1 = np.asarray(b1, dtype=np.float32)
    Wf = np.asarray(Wf, dtype=np.float32)
    bf = np.asarray(bf, dtype=np.float32)

    consts = _host_consts(W1, b1, Wf, bf)          # [128, COLS] bf16

    # x per core: [128(p), 8(c) * 104(bm)]: x[p, c*104+bm] =
    # e_output[core*8 + bm//13, bm%13, c*128+p]
    xs = (
        e_output[:, :A, :]
        .astype(bfloat16)
        .reshape(NCORES, BM, NCH, 128)
        .transpose(0, 3, 2, 1)
        .reshape(NCORES, 128, NCH * BM)
    )
    blobs = np.empty((NCORES, 128, COLS), bfloat16)
    blobs[:] = consts[None]
    blobs[:, :, XC:XC + NCH * BM] = xs

    if "nc" not in _COMPILED:
        _COMPILED["nc"] = build_program()
    nc = _COMPILED["nc"]

    in_maps = [{"blob": blobs[c]} for c in range(NCORES)]
    probe_b = [c * BPC for c in range(NCORES)]
    probe = _probe_batches(e_output, W1, b1, Wf, bf, probe_b)

    for attempt in range(3):
        bkr = run_bass_kernel_spmd(nc, in_maps, list(range(NCORES)))
        _COMPILED["last_results"] = bkr
        res = bkr.results

        out = np.empty((B, A, A, NB), np.float32)
        for c in range(NCORES):
            r = np.asarray(res[c]["out"])[:BPC * NB, :AA].astype(
                np.float32)                                 # [32, 169] rows 4b+n
            out[c * BPC:(c + 1) * BPC] = (
                r.reshape(BPC, NB, AA).transpose(0, 2, 1).reshape(BPC, A, A, NB)
            )
        # one host-recomputed probe batch per core guards against transient
        # device glitches; bf16 compute+output noise is ~3e-2 max-abs,
        # glitches are O(1)
        if np.abs(out[probe_b] - probe).max() < 8e-2:
            return out
    return out


if __name__ == "__main__":
    d = np.load("/root/problem/ref_cache.npz")
    got = kernel(
        e_output=d["e_output"], W1=d["W1"], b1=d["b1"], Wf=d["Wf"], bf=d["bf"],
        max_atoms=13,
    )
    exp = d["expected"]
    rel = np.linalg.norm(got - exp) / np.linalg.norm(exp)
    print("max abs err", np.abs(got - exp).max(), "rel", rel)


# revision 16
# speedup vs baseline: 1.0002x; 1.0002x over previous
"""Trainium2 Bass kernel for nn_EdgeClassify (gnn_message_passing), v2.

Reference computation (B=64, S=2048, D=1024, A=13, NB=4):
    red = einsum('bsd,ad->bsa', e_output, W1) + b1      # [B,S,A]
    f   = swapaxes(red[:, :A, :], 1, 2)                 # [B,A,A]  (only s<A used!)
    ga  = einsum('bia,na->bin', f, Wf[:, :A])
    gb  = einsum('bia,na->bin', f, Wf[:, A:])
    out[b,i,j,n] = ga[b,min(i,j),n] + gb[b,max(i,j),n] + bf[n], 0 on diagonal

Only e_output[:, :A, :] affects the output. Device math per core (8
batches, data parallel over B), all matmul operands bf16:
    Z  [104(b,m), 13(i)]  = sum_d x[(b,m), d] * W1[i, d]     (8 matmuls)
    G  [13(i), 64(s,b,n)] = Z.T @ W_blockdiag                (1 matmul)
    O  [32(b,n), 169(ij)] = Ga.T @ [M1T; CM] + Gb.T @ M2T    (2 matmuls,
                            accumulated; CM rows fold all b1/bf biases via
                            4 constant indicator rows in the lhsT)
PSUM->SBUF staging copies run on DVE (lowest post-op access latency of
the engines whose copies lower on this device path; gpsimd tensor_copy
and Activation copies both fail in this environment). Output goes out
via a SWDGE scatter-add whose descriptors are prepared during the input
DMA wait and fired with trigger_dma once the staging copy lands - this
skips both the 625ns HWDGE issue and the 650ns DGE->DMA delay on the
critical tail. The scatter moves bf16
(host converts back to fp32) into a pre-zeroed [32,256]-bf16 DRAM
buffer (256 = 169 padded to a 256B-multiple row stride; 240 declared
rows keep every idx value in bounds). The framework's entry/exit all-engine
barriers are stripped (every cross-engine dependency is explicitly
semaphored) and the first input DMA is hoisted above SP's entry branch,
together starting the input transfer ~270ns earlier.

Timeline (per core, TimelineSim): input DMA issue+latency 1275ns ->
x transfer 667ns -> DMA-sem prop 925ns -> mm1 -> Z copy -> mm2 ->
G copy -> mm3a/b -> out copy -> trigger+transfer ~80ns -> DMA-sem
prop 925ns. Total ~5805ns (baseline 8227ns).
"""

import os

import numpy as np

os.environ.setdefault("BASS_NEVER_TRACE", "1")

import concourse.bass as bass
import concourse.bacc as bacc
import concourse.mybir as mybir
from concourse.bass_utils import run_bass_kernel_spmd
from ml_dtypes import bfloat16

B, S, D, A, NB = 64, 2048, 1024, 13, 4
NCORES = 8
BPC = B // NCORES          # 8 batches per core
BM = BPC * A               # 104 (b, m) rows per core
AA = A * A                 # 169
NCH = D // 128             # 8 contraction chunks
OROW = 256                 # padded out row (bf16): 169 -> 256 (512B, 256B-aligned)
ODROWS = 240               # out DRAM rows: 32 used; padded so iota idx
                           # values (p + 16s, p<128) stay in bounds
F32 = mybir.dt.float32
BF16 = mybir.dt.bfloat16
I16 = mybir.dt.int16

# blob column offsets (bf16 columns)
W1C = 0                    # w1t: chunk c at cols c*13, row p = d%128
XC = NCH * A               # 104: x chunks (c-major, 104 cols each)
D1END = XC + NCH * BM      # 936: end of DMA1 (w1t + x only - critical path)
IDXC = D1END               # 936: scatter idx bits (2 cols, int16-as-bf16),
                           # in the off-critical consts DMA
WABC = IDXC + 2            # 938: block-diag [104, 64] both Wf halves
G2C = WABC + 64            # 1002: g2s lhsT [17, 64]; rows 13:17 host consts
M1C = G2C + 64             # 1066: [17, 169]: rows 0:13 M1T, 13:17 cm
M2C = M1C + AA             # 1235: [13, 169]: M2T
COLS = M2C + AA            # 1404
GR = A + NB                # 17: g2s rows (13 data + 4 bias indicators)

_COMPILED = {}


def build_program(out_mode="scatter", nwarm=7, warm_cols=256,
                  copy_eng="dve", final_wait=True, act_split=False,
                  strip_barriers=True) -> bass.Bass:
    nc = bacc.Bacc("TRN2", target_bir_lowering=False, debug=False,
                   num_devices=NCORES)

    blob_d = nc.declare_dram_parameter("blob", [128, COLS], BF16, isOutput=False)
    out_d = nc.declare_dram_parameter("out", [ODROWS, OROW], BF16, isOutput=True)

    from contextlib import ExitStack
    with ExitStack() as es:
        blob = es.enter_context(nc.sbuf_tensor([128, COLS], BF16))
        zs = es.enter_context(nc.sbuf_tensor([BM, A], BF16))
        idxt = es.enter_context(nc.sbuf_tensor([128, 2], I16))
        outs = es.enter_context(nc.sbuf_tensor([128, 1, OROW], BF16))
        wp = es.enter_context(nc.psum_tensor([1, warm_cols], F32))
        zp = es.enter_context(nc.psum_tensor([BM, A], F32))
        gp = es.enter_context(nc.psum_tensor([A, 64], F32))
        op = es.enter_context(nc.psum_tensor([BPC * NB, AA], F32))
        (dsem1, dsem2, zsem, dsem3, pm, psem, isem, s1, sza, s2, sc, s3,
         sv) = (es.enter_context(nc.semaphore(n)) for n in (
            "dsem1", "dsem2", "zsem", "dsem3", "pm", "psem", "isem", "s1",
            "sza", "s2", "sc", "s3", "sv"))
        block = es.enter_context(nc.Block())
        @block.sync
        def _(sync):
            # w1t + x + scatter idx first: gates stage 1 (and the scatter
            # prep); consts transfer while stage 1's data is still in flight
            sync.dma_start(blob[:, 0:D1END], blob_d[:, 0:D1END]).then_inc(
                dsem1, 16)
            sync.dma_start(blob[:, D1END:COLS], blob_d[:, D1END:COLS]).then_inc(
                dsem2, 16)
            if out_mode in ("scatter", "scatter_direct"):
                # pre-zero the DRAM output (scatter-add needs a clean base).
                # Source rows 32:64 (memset zeros the whole tile): the copies
                # later write rows 0:32, so this read never conflicts with
                # them and they need no zsem ordering.
                sync.dma_start(out_d[0:BPC * NB, :], outs[BPC * NB:2 * BPC * NB,
                                                          0, :]
                               ).wait_op(pm, 1, "sem-ge").then_inc(zsem, 16)
            else:
                sync.dma_start(out_d[0:BPC * NB, :], outs[0:BPC * NB, 0, :]
                               ).wait_op(sv, 2 if act_split else 1,
                                         "sem-ge").then_inc(dsem3, 16)

        @block.gpsimd
        def _(gpsimd):
            if out_mode == "scatter" and copy_eng in ("pool", "pool_blobidx"):
                if copy_eng == "pool":
                    # idx on-device (p + 16s): frees the scatter prep from
                    # the input-DMA wait, so Pool's engine is idle in time
                    # for the PSUM->SBUF staging copies below
                    nc.gpsimd.iota(idxt[:, :], pattern=[[16, 2]], base=0,
                                   channel_multiplier=1).then_inc(isem, 1)
                    prep_wait, prep_val = isem, 1
                    idxs_ap = idxt[:, :]
                else:
                    prep_wait, prep_val = dsem2, 16
                    idxs_ap = blob[0:128, IDXC:IDXC + 2].bitcast(I16)
                nc.gpsimd.dma_scatter_add(
                    out_ap=out_d[:, :],
                    in_ap=outs[:, :, :],
                    idxs_ap=idxs_ap,
                    num_idxs=BPC * NB,
                    num_idxs_reg=BPC * NB,
                    elem_size=OROW,
                    prepare_only=True,
                    sem=dsem3,
                ).wait_op(prep_wait, prep_val, "sem-ge").then_inc(psem, 1)
                gpsimd.memset(outs[:, :, :], 0.0).then_inc(pm, 1)
                # staging copies: gpsimd has no post-op access latency (vs
                # DVE's +125ns) and the trigger below waits on a same-engine
                # semaphore
                nc.gpsimd.tensor_copy(zs[:], zp[:]).wait_op(
                    s1, 1, "sem-ge").then_inc(sza, 1)
                nc.gpsimd.tensor_copy(blob[0:A, G2C:G2C + 64], gp[:]).wait_op(
                    s2, 1, "sem-ge").then_inc(sc, 1)
                gpsimd.wait_ge(zsem, 16)
                nc.gpsimd.tensor_copy(outs[0:BPC * NB, 0, 0:AA], op[:]).wait_op(
                    s3, 1, "sem-ge").then_inc(sv, 1)
                gpsimd.wait_ge(psem, 1)
                nc.gpsimd.trigger_dma(count=1).wait_op(sv, 1, "sem-ge")
            elif out_mode == "scatter":
                gpsimd.memset(outs[:, :, :], 0.0).then_inc(pm, 1)
                nc.gpsimd.dma_scatter_add(
                    out_ap=out_d[:, :],
                    in_ap=outs[:, :, :],
                    idxs_ap=blob[0:128, IDXC:IDXC + 2].bitcast(I16),
                    num_idxs=BPC * NB,
                    num_idxs_reg=BPC * NB,
                    elem_size=OROW,
                    prepare_only=True,
                    sem=dsem3,
                ).wait_op(dsem2, 16, "sem-ge").then_inc(psem, 1)
                gpsimd.wait_ge(psem, 1)
                gpsimd.wait_ge(zsem, 16)
                nc.gpsimd.trigger_dma(count=1).wait_op(sv, 2 if act_split else 1, "sem-ge")
            elif out_mode == "scatter_direct":
                gpsimd.memset(outs[:, :, :], 0.0).then_inc(pm, 1)
                gpsimd.wait_ge(zsem, 16)
                nc.gpsimd.dma_scatter_add(
                    out_ap=out_d[:, :],
                    in_ap=outs[:, :, :],
                    idxs_ap=blob[0:128, IDXC:IDXC + 2].bitcast(I16),
                    num_idxs=BPC * NB,
                    num_idxs_reg=BPC * NB,
                    elem_size=OROW,
                ).wait_op(sv, 2 if act_split else 1, "sem-ge").then_inc(dsem3, 16)

        @block.tensor
        def _(tensor):
            # warm-up matmuls on (garbage) blob data keep the PE p-state
            # ramped while the input DMA is in flight
            for _ in range(nwarm):
                nc.tensor.matmul(wp[:], blob[:, 0:1], blob[:, 0:warm_cols],
                                 start=True, stop=True)
            # stage 1: Z[(b,m), i] = sum_d x[(b,m), d] * W1[i, d]
            for c in range(NCH):
                mm = nc.tensor.matmul(
                    zp[:],
                    blob[:, XC + c * BM:XC + (c + 1) * BM],  # lhsT [128, 104]
                    blob[:, W1C + c * A:W1C + (c + 1) * A],  # rhs  [128, 13]
                    start=(c == 0),
                    stop=(c == NCH - 1),
                )
                if c == 0:
                    mm.wait_op(dsem1, 16, "sem-ge")
            mm.then_inc(s1, 1)
            # stage 2: G[i, (side,b,n)] = Z.T @ W_blockdiag(both halves)
            # (consts wait is standalone: dsem2 fires well before sza)
            tensor.wait_ge(dsem2, 16)
            nc.tensor.matmul(
                gp[:], zs[:], blob[0:BM, WABC:WABC + 64],
                start=True, stop=True,
            ).wait_op(sza, 1, "sem-ge").then_inc(s2, 1)
            # stage 3: O = Ga.T @ [M1T; CM] + Gb.T @ M2T  (accumulate in op)
            nc.tensor.matmul(
                op[:], blob[0:GR, G2C:G2C + 32], blob[0:GR, M1C:M1C + AA],
                start=True, stop=False, skip_group_check=True,
            ).wait_op(sc, 1, "sem-ge")
            nc.tensor.matmul(
                op[:], blob[0:A, G2C + 32:G2C + 64], blob[0:A, M2C:M2C + AA],
                start=False, stop=True, skip_group_check=True,
            ).then_inc(s3, 1)

        # out-copy column split: DVE takes cols 0:OSPL, Act takes the rest;
        # tuned so both engines' (processing + access-ack) latencies finish
        # together, ~30ns sooner than DVE alone. Small copies stay DVE-only
        # (Act's 370ns access init dwarfs them).
        OSPL = 139 if act_split else AA

        if copy_eng == "dve":
            @block.vector
            def _(vector):
                nc.vector.tensor_copy(zs[:], zp[:]).wait_op(
                    s1, 1, "sem-ge").then_inc(sza, 1)
                nc.vector.tensor_copy(blob[0:A, G2C:G2C + 64], gp[:]).wait_op(
                    s2, 1, "sem-ge").then_inc(sc, 1)
                # order the outs write after Pool's memset (fires ~340ns in)
                if out_mode == "scatter":
                    vector.wait_ge(pm, 1)
                nc.vector.tensor_copy(outs[0:BPC * NB, 0, 0:OSPL],
                                      op[:, 0:OSPL]).wait_op(
                    s3, 1, "sem-ge").then_inc(sv, 1)

            if act_split:
                @block.scalar
                def _(scalar):
                    # order the outs write after Pool's memset (fires ~340ns
                    # in, long before this engine's act-table load completes)
                    if out_mode == "scatter":
                        scalar.wait_ge(pm, 1)
                    nc.scalar.copy(outs[0:BPC * NB, 0, OSPL:AA],
                                   op[:, OSPL:AA]).wait_op(
                        s3, 1, "sem-ge").then_inc(sv, 1)

    if final_wait:
        # SP EventSemaphore costs 25ns after the sem resolves; cheaper
        # waiters don't exist (a no-op trigger_dma would be 0-cost in the
        # model but the executor/ucode reject an empty-FIFO trigger)
        nc.sync.wait_ge(dsem3, 16)

    _strip_dead_const_inits(nc)
    if strip_barriers:
        _strip_barriers(nc)
    _hoist_first_dma(nc)
    nc.finalize()
    return nc


def _hoist_first_dma(nc):
    """Move SP's first DMACopy from its body block into `main`, ahead of the
    UnconditionalBranch, so the input DMA issues ~50ns earlier."""
    import concourse.mybir as mb
    fn = nc.m.functions[0]
    blocks = {b.name: b for b in fn.blocks}
    main = fn.blocks[0]
    sp = mb.EngineType.SP
    br_i = next((k for k, i in enumerate(main.instructions)
                 if i.engine == sp
                 and type(i).__name__ == "InstUnconditionalBranch"), None)
    if br_i is None:
        return
    target = main.instructions[br_i].target
    body = blocks.get(target)
    if body is None or not body.instructions:
        return
    first = body.instructions[0]
    if type(first).__name__ != "InstDMACopy" or first.engine != sp:
        return
    body.instructions = body.instructions[1:]
    main.instructions = (main.instructions[:br_i] + [first]
                         + main.instructions[br_i:])


def _strip_barriers(nc):
    """Remove the framework's entry/exit all-engine barriers (Drain +
    barrier_* EventSemaphore per engine). Every cross-engine dependency in
    this program is ordered by an explicit semaphore, so the barriers only
    delay the first DMA by ~220ns. Exit Drains are also dropped; the final
    SP wait on the output-DMA semaphore keeps the program alive."""
    barrier_sems = set()
    for name, inst in nc.inst_map.items():
        if name.startswith("barrier_"):
            si = getattr(inst, "sync_info", None)
            if si is not None:
                for w in (si.on_wait or []):
                    barrier_sems.add(w.id)
                for u in (si.on_update or []):
                    barrier_sems.add(u.id)
    dead = set()
    for name, inst in nc.inst_map.items():
        tname = type(inst).__name__
        if name.startswith("barrier_"):
            dead.add(name)
        elif tname == "InstDrain":
            si = getattr(inst, "sync_info", None)
            refs = set()
            if si is not None:
                refs = {w.id for w in (si.on_wait or [])} | {
                    u.id for u in (si.on_update or [])}
            if refs <= barrier_sems:
                dead.add(name)
    if not dead:
        return
    for f in nc.m.functions:
        for b in f.blocks:
            b.instructions = [i for i in b.instructions if i.name not in dead]


def _strip_dead_const_inits(nc):
    """Drop preamble memsets that initialize Bass's lazy scratch constants
    when nothing in the program reads them (starts the first DMA earlier)."""
    read = set()
    inits = {}
    for name, inst in nc.inst_map.items():
        for ap in (getattr(inst, "ins", None) or []):
            mr = getattr(ap, "memref", "")
            if isinstance(mr, str) and mr.startswith("const-"):
                read.add(mr)
        if type(inst).__name__ == "InstMemset":
            outs = getattr(inst, "outs", None)
            if outs:
                mr = getattr(outs[0], "memref", "")
                if isinstance(mr, str) and mr.startswith("const-"):
                    inits.setdefault(mr, []).append(name)
    dead = {n for mr, names in inits.items() if mr not in read for n in names}
    if not dead:
        return
    for f in nc.m.functions:
        for b in f.blocks:
            b.instructions = [i for i in b.instructions if i.name not in dead]


def _host_consts(W1, b1, Wf, bf):
    """Host-precomputed constant blob columns (everything except x)."""
    Wa, Wb = Wf[:, :A], Wf[:, A:]
    cb = np.zeros((128, COLS), np.float32)

    # w1t: chunk c at cols c*13: w1t[p, c*13+i] = W1[i, c*128+p]
    cb[:, W1C:W1C + NCH * A] = (
        W1.T.reshape(NCH, 128, A).transpose(1, 0, 2).reshape(128, NCH * A)
    )

    # wab block-diag [104, 64]: rows (b,m), cols side*32 + b*4 + n
    for b in range(BPC):
        cb[b * A:(b + 1) * A, WABC + b * NB:WABC + (b + 1) * NB] = Wa.T
        cb[b * A:(b + 1) * A,
           WABC + 32 + b * NB:WABC + 32 + (b + 1) * NB] = Wb.T

    # g2s const rows 13:17: indicator [n == k] at col side*32 + b*4 + n
    for k in range(NB):
        for side in range(2):
            for b in range(BPC):
                cb[A + k, G2C + side * 32 + b * NB + k] = 1.0

    idx = np.arange(A)
    I, J = np.meshgrid(idx, idx, indexing="ij")
    offd = (I != J).astype(np.float32).reshape(-1)
    mn, mx = np.minimum(I, J).reshape(-1), np.maximum(I, J).reshape(-1)
    m1t = np.zeros((A, AA), np.float32)
    m2t = np.zeros((A, AA), np.float32)
    cols = np.arange(AA)
    m1t[mn, cols] = offd
    m2t[mx, cols] = offd
    cb[0:A, M1C:M1C + AA] = m1t
    cb[0:A, M2C:M2C + AA] = m2t
    # cm rows 13:17 of the M1 weight: fold b1/bf biases
    sa, sb = Wa.sum(1), Wb.sum(1)
    cm = (bf[:, None] + np.outer(sa, b1[mn]) + np.outer(sb, b1[mx])) * offd[None, :]
    cb[A:GR, M1C:M1C + AA] = cm

    cbf = cb.astype(bfloat16)

    # scatter idx: [128, 2] int16, idx j at [j%16, j//16]. Only rows 0:16
    # are decoded; pad rows with 0 (in-bounds, and 0x0000 is not a bf16 NaN,
    # which -1 = 0xFFFF would be)
    idx16 = np.zeros((128, 2), np.int16)
    for j in range(BPC * NB):
        idx16[j % 16, j // 16] = j
    cbf[:, IDXC:IDXC + 2] = idx16.view(bfloat16)
    return cbf


def _probe_batches(e_output, W1, b1, Wf, bf, batches):
    """Host-side fp32 recompute of whole batches - guards against transient
    device glitches (O(1) corruption; bf16 noise is ~5e-3)."""
    Wa, Wb = Wf[:, :A], Wf[:, A:]
    wab = np.concatenate([Wa, Wb], axis=0).T                  # [13, 8]
    idx = np.arange(A)
    I, J = np.meshgrid(idx, idx, indexing="ij")
    offd = (I != J).astype(np.float32).reshape(-1)
    mn, mx = np.minimum(I, J).reshape(-1), np.maximum(I, J).reshape(-1)
    m1t = np.zeros((A, AA), np.float32)
    m2t = np.zeros((A, AA), np.float32)
    cols = np.arange(AA)
    m1t[mn, cols] = offd
    m2t[mx, cols] = offd
    sa, sb = Wa.sum(1), Wb.sum(1)
    cm = (bf[:, None] + np.outer(sa, b1[mn]) + np.outer(sb, b1[mx])) * offd[None, :]
    out = np.empty((len(batches), A, A, NB), np.float32)
    for k, b in enumerate(batches):
        zb = e_output[b, :A, :] @ W1.T                        # [13(m), 13(i)]
        g = zb.T @ wab                                        # [13(i), 8]
        ob = g[:, :NB].T @ m1t + g[:, NB:].T @ m2t + cm       # [4, 169]
        out[k] = ob.T.reshape(A, A, NB)
    return out


def kernel(e_output, W1, b1, Wf, bf, max_atoms):
    assert int(max_atoms) == A
    e_output = np.asarray(e_output, dtype=np.float32)
    W1 = np.asarray(W1, dtype=np.float32)
    b1 = np.asarray(b1, dtype=np.float32)
    Wf = np.asarray(Wf, dtype=np.float32)
    bf = np.asarray(bf, dtype=np.float32)

    consts = _host_consts(W1, b1, Wf, bf)          # [128, COLS] bf16

    # x per core: [128(p), 8(c) * 104(bm)]: x[p, c*104+bm] =
    # e_output[core*8 + bm//13, bm%13, c*128+p]
    xs = (
        e_output[:, :A, :]
        .astype(bfloat16)
        .reshape(NCORES, BM, NCH, 128)
        .transpose(0, 3, 2, 1)
        .reshape(NCORES, 128, NCH * BM)
    )
    blobs = np.empty((NCORES, 128, COLS), bfloat16)
    blobs[:] = consts[None]
    blobs[:, :, XC:XC + NCH * BM] = xs

    if "nc" not in _COMPILED:
        _COMPILED["nc"] = build_program()
    nc = _COMPILED["nc"]

    in_maps = [{"blob": blobs[c]} for c in range(NCORES)]
    probe_b = [c * BPC for c in range(NCORES)]
    probe = _probe_batches(e_output, W1, b1, Wf, bf, probe_b)

    for attempt in range(3):
        bkr = run_bass_kernel_spmd(nc, in_maps, list(range(NCORES)))
        _COMPILED["last_results"] = bkr
        res = bkr.results

        out = np.empty((B, A, A, NB), np.float32)
        for c in range(NCORES):
            r = np.asarray(res[c]["out"])[:BPC * NB, :AA].astype(
                np.float32)                                 # [32, 169] rows 4b+n
            out[c * BPC:(c + 1) * BPC] = (
                r.reshape(BPC, NB, AA).transpose(0, 2, 1).reshape(BPC, A, A, NB)
            )
        # one host-recomputed probe batch per core guards against transient
        # device glitches; bf16 compute+output noise is ~3e-2 max-abs,
        # glitches are O(1)
        if np.abs(out[probe_b] - probe).max() < 8e-2:
            return out
    return out


if __name__ == "__main__":
    d = np.load("/root/problem/ref_cache.npz")
    got = kernel(
        e_output=d["e_output"], W1=d["W1"], b1=d["b1"], Wf=d["Wf"], bf=d["bf"],
        max_atoms=13,
    )
    exp = d["expected"]
    rel = np.linalg.norm(got - exp) / np.linalg.norm(exp)
    print("max abs err", np.abs(got - exp).max(), "rel", rel)
